# revision 1
# baseline (speedup 1.0000x reference)
"""Trainium2 Bass kernel for nn_BasicNCAModel (neural cellular automaton).

Model (per step, 4 steps):
  y = concat([x, dwconv3x3(x, f1), dwconv3x3(x, f2)])   (reflect pad)
  dx = relu(y @ w1 + b1) @ w2
  x  = x + dx * (stoch > 0.5) * ch_mask

Kernel strategy:
  - Pure data parallel: batch 16 -> 2 samples on each of 8 NeuronCores.
  - Channel-major layout [C=32, H, W]; the depthwise convs + first dense
    layer fold into a single 3x3 conv with effective weights
    W_eff[dy,dx] = diag(f1[dy,dx]) @ w1[32:64] + diag(f2[dy,dx]) @ w1[64:96]
    (+ w1[0:32] at the center tap). Per 512-pixel tile this is 6 matmuls
    (3 horizontal taps x 2 output halves of 256) with K=96 = 3 vertically
    shifted copies of x stacked on partitions; horizontal taps are free-dim
    AP offsets. The bias rides as a 97th ones-row on the center tap.
    Second layer: 2 matmuls K=128. ch_mask is folded into w2 (cols 0..2
    zeroed), so the residual add is exact for the image channels.
  - Matmul operands are fp16 (full PE rate + fast weight load; ~2^-11
    rounding like fp32r but without its half-rate 2-pass behavior).
    The residual add x + dx*mask runs in exact fp32 on the vector engine
    from a separate fp32 load of the band interior.
  - x lives in DRAM column-padded [C, H, W+2] so band loads/stores are
    fully contiguous per partition; reflect rows are handled by DMA
    segmenting, reflect columns by two tiny on-chip copies. x ping-pongs
    between two internal DRAM buffers across the 4 steps.
"""

import numpy as np
from contextlib import ExitStack

import concourse.bacc as bacc
import concourse.tile as tile
from concourse import mybir
from concourse.bass_utils import run_bass_kernel_spmd

F32 = mybir.dt.float32
F16 = mybir.dt.float16
AF = mybir.ActivationFunctionType
OP = mybir.AluOpType

B, C, H, W = 16, 32, 256, 256
IMG = 3
FIRE = 0.5
NCORES = 8
BPC = B // NCORES          # samples per core = 2
BR = 16                    # band rows
NB = H // BR               # bands per sample = 16
ROWS_PER_TILE = 2          # 2 rows x 256 cols = 512-pixel matmul tiles
TPB = BR // ROWS_PER_TILE  # tiles per band = 8
NSTEP = 4
WP = W + 2                 # padded row length 258


def _seg_rows(r0: int, dy: int):
    """Contiguous (src_row, dst_row, n) segments for one vertical copy,
    with reflect handling at the image top/bottom (reflect: -1->1, 256->254)."""
    rows = [r0 + dy + i for i in range(BR)]
    refl = [(-r if r < 0 else (2 * (H - 1) - r if r > H - 1 else r)) for r in rows]
    segs = []
    i = 0
    while i < BR:
        j = i + 1
        while j < BR and refl[j] == refl[i] + (j - i):
            j += 1
        segs.append((refl[i], i, j - i))
        i = j
    return segs


def _build():
    nc = bacc.Bacc("TRN2", target_bir_lowering=False, debug=False,
                   num_devices=NCORES)
    xin = nc.dram_tensor("xin", [BPC, C, H, WP], F32, kind="ExternalInput").ap()
    stoch = nc.dram_tensor("stoch", [NSTEP, BPC, H, W], F32,
                           kind="ExternalInput").ap()
    wm = nc.dram_tensor("wm", [96, 256], F16, kind="ExternalInput").ap()
    w0 = nc.dram_tensor("w0", [97, 256], F16, kind="ExternalInput").ap()
    wp = nc.dram_tensor("wp", [96, 256], F16, kind="ExternalInput").ap()
    w2h = nc.dram_tensor("w2h", [128, 64], F16, kind="ExternalInput").ap()
    yout = nc.dram_tensor("y", [BPC, C, H, WP], F32, kind="ExternalOutput").ap()

    with tile.TileContext(nc) as tc, ExitStack() as ctx:
        dram = ctx.enter_context(tc.tile_pool(name="dram", bufs=1, space="DRAM"))
        xA = dram.tile([BPC, C, H, WP], F32, name="xA")
        xB = dram.tile([BPC, C, H, WP], F32, name="xB")

        wpool = ctx.enter_context(tc.tile_pool(name="wpool", bufs=1))
        wmt = wpool.tile([96, 256], F16, name="wmt")
        w0t = wpool.tile([97, 256], F16, name="w0t")
        wpt = wpool.tile([96, 256], F16, name="wpt")
        w2t = wpool.tile([128, 64], F16, name="w2t")
        ones = wpool.tile([1, BR * WP], F16, name="ones")
        nc.sync.dma_start(wmt[:], wm)
        nc.sync.dma_start(w0t[:], w0)
        nc.sync.dma_start(wpt[:], wp)
        nc.sync.dma_start(w2t[:], w2h)
        nc.gpsimd.memset(ones[:], 1.0)

        xt_pool = ctx.enter_context(tc.tile_pool(name="xt", bufs=4))
        xc_pool = ctx.enter_context(tc.tile_pool(name="xc", bufs=2))
        st_pool = ctx.enter_context(tc.tile_pool(name="st", bufs=2))
        stb_pool = ctx.enter_context(tc.tile_pool(name="stb", bufs=2))
        xn_pool = ctx.enter_context(tc.tile_pool(name="xn", bufs=2))
        hs_pool = ctx.enter_context(tc.tile_pool(name="hs", bufs=3))
        dxm_pool = ctx.enter_context(tc.tile_pool(name="dxm", bufs=3))
        hp_pool = ctx.enter_context(tc.tile_pool(name="hp", bufs=3, space="PSUM"))
        dxp_pool = ctx.enter_context(tc.tile_pool(name="dxp", bufs=2, space="PSUM"))

        srcs = [xin, xA[:], xB[:], xA[:]]
        dsts = [xA[:], xB[:], xA[:], yout]

        for step in range(NSTEP):
            src, dst = srcs[step], dsts[step]
            for s in range(BPC):
                for b in range(NB):
                    r0 = b * BR
                    # ---- load: 3 vertically shifted fp16 copies of the band.
                    # partition groups: 0-31 dy=0 (center), 32-63 dy=-1,
                    # 64-95 dy=+1 — center first so the residual/mask ops all
                    # share base partition 0 (DVE needs equal base partitions).
                    xt = xt_pool.tile([97, BR * WP], F16)
                    xtr = xt[:].rearrange("p (r c) -> p r c", c=WP)
                    for gi, dy in enumerate((0, -1, 1)):
                        p0 = gi * 32
                        for (sr, dr, n) in _seg_rows(r0, dy):
                            # SWDGE load, cast f32 -> fp16 in flight (cheap
                            # trigger; descriptor gen runs on Q7 cores, off
                            # the engine queues); contiguous [n*WP]/channel
                            nc.gpsimd.dma_start(
                                xtr[p0:p0 + 32, dr:dr + n, :],
                                src[s, :, sr:sr + n, :])
                    # ones row for the bias (97th K row of the center tap)
                    nc.gpsimd.dma_start(xt[96:97, :], ones[:])
                    # reflect column pads: col0 <- col2, col257 <- col255
                    nc.vector.tensor_copy(xtr[0:96, :, 0:1], xtr[0:96, :, 2:3])
                    nc.vector.tensor_copy(xtr[0:96, :, WP - 1:WP],
                                          xtr[0:96, :, WP - 3:WP - 2])

                    # exact fp32 copy of the band for the residual add
                    xc = xc_pool.tile([32, BR * WP], F32)
                    xcr = xc[:].rearrange("p (r c) -> p r c", c=WP)
                    nc.gpsimd.dma_start(xc[:], src[s, :, r0:r0 + BR, :]
                                        .rearrange("p r c -> p (r c)"))

                    # ---- stochastic values, broadcast across channels ----
                    st = st_pool.tile([1, BR * W], F32)
                    nc.gpsimd.dma_start(
                        st[:], stoch[step, s, r0:r0 + BR, :].flatten().unsqueeze(0))
                    stb = stb_pool.tile([32, BR * W], F32)
                    nc.gpsimd.partition_broadcast(stb[:], st[:])
                    stbr = stb[:].rearrange("p (r c) -> p r c", c=W)

                    xn = xn_pool.tile([32, BR * WP], F32)
                    xnr = xn[:].rearrange("p (r c) -> p r c", c=WP)
                    # pad columns are stored to DRAM but never consumed as
                    # data; init them so the contiguous store reads defined
                    # memory (single strided memset covers cols 0 and 257)
                    nc.gpsimd.memset(xnr[:, :, 0:WP:WP - 1], 0.0)

                    # software pipeline: layer 2 of tile t-1 is emitted after
                    # layer 1 of tile t, giving the relu a full L1-block of
                    # slack before the PE needs its output
                    pend = None  # (hs, rt) awaiting layer 2
                    for t in range(TPB + 1):
                        if t < TPB:
                            rt = t * ROWS_PER_TILE
                            # ---- layer 1: 3x3 conv (3 taps x 2 halves) ----
                            hp = hp_pool.tile([128, 1024], F32)
                            for h in range(2):
                                out = hp[:, h * 512:(h + 1) * 512]
                                hslc = slice(h * 128, (h + 1) * 128)
                                nc.tensor.matmul(
                                    out, wmt[:, hslc],
                                    xtr[0:96, rt:rt + ROWS_PER_TILE, 0:W],
                                    start=True, stop=False)
                                nc.tensor.matmul(
                                    out, w0t[:, hslc],
                                    xtr[0:97, rt:rt + ROWS_PER_TILE, 1:W + 1],
                                    start=False, stop=False)
                                nc.tensor.matmul(
                                    out, wpt[:, hslc],
                                    xtr[0:96, rt:rt + ROWS_PER_TILE, 2:W + 2],
                                    start=False, stop=True)
                            # ---- relu (bias already added via ones row) ----
                            hs = hs_pool.tile([128, 1024], F16)
                            nc.scalar.activation(hs[:], hp[:], AF.Relu)
                            pend, prev = (hs, rt), pend
                        else:
                            prev, pend = pend, None
                        if prev is None:
                            continue
                        hs_p, rp = prev
                        # ---- layer 2: dx = h @ w2 (K=256 split in two) ----
                        dxp = dxp_pool.tile([32, 512], F32)
                        nc.tensor.matmul(dxp[:], w2t[:, 0:32], hs_p[:, 0:512],
                                         start=True, stop=False)
                        nc.tensor.matmul(dxp[:], w2t[:, 32:64], hs_p[:, 512:1024],
                                         start=False, stop=True)
                        # ---- masked residual: xn = (stoch>0.5)*dx + x ----
                        dxpr = dxp[:].rearrange("p (r c) -> p r c", c=W)
                        dxm = dxm_pool.tile([32, ROWS_PER_TILE * W], F32)
                        dxmr = dxm[:].rearrange("p (r c) -> p r c", c=W)
                        nc.vector.scalar_tensor_tensor(
                            dxmr, stbr[:, rp:rp + ROWS_PER_TILE, :], FIRE,
                            dxpr, op0=OP.is_gt, op1=OP.mult)
                        nc.vector.tensor_add(
                            xnr[:, rp:rp + ROWS_PER_TILE, 1:W + 1], dxmr,
                            xcr[:, rp:rp + ROWS_PER_TILE, 1:W + 1])

                    # ---- store band (contiguous, pads included) ----
                    nc.sync.dma_start(
                        dst[s, :, r0:r0 + BR, :].rearrange("p r c -> p (r c)"),
                        xn[:])
    nc.compile()
    return nc


_NC_CACHE = None


def _get_nc():
    global _NC_CACHE
    if _NC_CACHE is None:
        _NC_CACHE = _build()
    return _NC_CACHE


def _make_in_maps(x, f1, f2, w1, b1, w2, stoch):
    f1 = np.asarray(f1, np.float64)[:, :, 0, :]   # [3,3,32]
    f2 = np.asarray(f2, np.float64)[:, :, 0, :]
    w1 = np.asarray(w1, np.float64)               # [96,256]
    b1 = np.asarray(b1, np.float64)               # [256]
    w2 = np.asarray(w2, np.float64).copy()        # [256,32]
    w2[:, :IMG] = 0.0                             # ch_mask folded into w2

    # W_eff[dy,dx][c,:] = f1[dy,dx,c]*w1[32+c,:] + f2[dy,dx,c]*w1[64+c,:]
    #                     (+ w1[c,:] at the center tap)
    weff = (f1[:, :, :, None] * w1[None, None, 32:64, :]
            + f2[:, :, :, None] * w1[None, None, 64:96, :])   # [3,3,32,256]
    weff[1, 1] += w1[0:32, :]

    def col(dxi):  # stack the 3 vertical taps along K for horizontal tap dxi
        # row order matches xt partition groups: dy=0, dy=-1, dy=+1
        return np.concatenate([weff[1, dxi], weff[0, dxi], weff[2, dxi]], axis=0)

    wm = col(0).astype(np.float16)                                    # [96,256]
    w0 = np.concatenate([col(1), b1[None, :]], axis=0).astype(np.float16)
    wpm = col(2).astype(np.float16)                                   # [96,256]
    w2h = np.concatenate([w2[0:128, :], w2[128:256, :]], axis=1).astype(np.float16)

    x = np.asarray(x, np.float32)
    stoch = np.asarray(stoch, np.float32)
    in_maps = []
    for i in range(NCORES):
        xi = np.transpose(x[i * BPC:(i + 1) * BPC], (0, 3, 1, 2))  # [2,32,H,W]
        xpad = np.zeros((BPC, C, H, WP), np.float32)
        xpad[:, :, :, 1:W + 1] = xi
        sti = np.ascontiguousarray(
            stoch[:, i * BPC:(i + 1) * BPC, :, :, 0])
        in_maps.append({"xin": xpad, "stoch": sti, "wm": wm, "w0": w0,
                        "wp": wpm, "w2h": w2h})
    return in_maps


def kernel(x, f1, f2, w1, b1, w2, stoch, steps):
    assert int(steps) == NSTEP, f"kernel compiled for {NSTEP} steps, got {steps}"
    nc = _get_nc()
    in_maps = _make_in_maps(x, f1, f2, w1, b1, w2, stoch)
    res = run_bass_kernel_spmd(nc, in_maps, core_ids=list(range(NCORES)))
    outs = []
    for i in range(NCORES):
        yi = res.results[i]["y"][:, :, :, 1:W + 1]     # strip col pads
        outs.append(np.transpose(yi, (0, 2, 3, 1)))    # -> [2,256,256,32]
    return np.ascontiguousarray(np.concatenate(outs, axis=0)).astype(np.float32)

